# revision 31
# baseline (speedup 1.0000x reference)
"""Multi-head attention (B=4, S=2048, d_model=1024, H=16) on 8 TRN2 NeuronCores.

Sharding: tensor-parallel over heads x data-parallel over batch.
Core c handles batch b=c//2 and head group g=c%2 (8 heads = 512 of the
1024 d_model columns of W_Q/W_K/W_V, and 512 rows of W_O). Each core
produces a partial output Y_partial[b] = O_g @ W_O[g-rows, :]; the host
sums the two partials per batch.

Device-side dataflow per core (all matmul operands fp16, accum fp32):
  - log2e/8 is folded into W_Q on the host, so scores arrive in the
    log2 domain: exp(s/8) == 2^u with u the raw matmul output
  - k^T, q^T = W^T X^T         (lhsT = W chunk, rhs = X^T chunk)
  - v = X @ W_V   in [token, head-dim] layout, with a ones column
  - per head pair, per 128-ktok block: scores^T = k^T.T q^T -> PSUM
    (row-tiled 64x128 pair, concurrent in the PE array)
    2^u -> P^T fp16 via Act-engine exp (scale=ln2) for 3 of 4 blocks,
    and via a single DVE tensor_scalar Schraudolph (int16 bit trick)
    for the 4th -- splits the exp load across both engines
    out^T_ext += [v_h | 1].T @ P^T   (row 64 = softmax denominator)
  - out^T / denominator -> O^T (reciprocal_approx_fast + gpsimd bcast)
  - Y_partial = O @ W_O slice -> DRAM fp16, summed on host

Scheduling: projections are emitted with per-iteration deadlines into
the attention stream (earliest exp at ~13us instead of ~70us), and the
output projection is emitted eagerly inside the last pair's qb loop.
"""

import math
import numpy as np

B = 4
S = 2048
D = 1024
H = 16
DK = 64
NCORES = 8
HPC = 8          # heads per core
GCOLS = 512      # d_model columns per head group
QB = 512         # q-token block (PSUM bank free dim)
NQB = S // QB    # 4
NKB = S // 128   # 16 k-token blocks
NC_CHUNKS = D // 128  # 8 contraction chunks

LOG2E = math.log2(math.e)
LN2 = math.log(2.0)
# fp16 Schraudolph: j = round(1024*u + (15*1024 - C)); bits(j) ~ 2^u
SCHRAU_BIAS = float(15 * 1024 - 60)
# which kb iterations run exp on DVE instead of Act (1 of 4 = 25%)
import os
DVE_KB = (frozenset((3, 7, 11, 15)) if os.environ.get("NO_DVE_EXP") != "1"
          else frozenset())
FAST_RECIP = os.environ.get("NO_FAST_RECIP") != "1"

_prog_cache = {}


def build_program(reps=1):
    key = (reps,)
    if key in _prog_cache:
        return _prog_cache[key]

    import concourse.bacc as bacc
    import concourse.mybir as mybir
    from concourse.tile import TileContext

    f16 = mybir.dt.float16
    i16 = mybir.dt.int16
    f32 = mybir.dt.float32
    EXP = mybir.ActivationFunctionType.Exp
    MULT = mybir.AluOpType.mult
    ADD = mybir.AluOpType.add

    nc = bacc.Bacc("TRN2", target_bir_lowering=False, debug=False,
                   num_devices=NCORES)

    # DRAM parameters (per-core shards, pre-laid-out on host)
    # token-block-major for kt/qt, kb-major for vt => in-order small DMAs
    qt_d = nc.dram_tensor("qt", [NQB, 128, NC_CHUNKS, QB], f16,
                          kind="ExternalInput").ap()
    kt_d = nc.dram_tensor("kt", [NQB, 128, NC_CHUNKS, QB], f16,
                          kind="ExternalInput").ap()
    vt_d = nc.dram_tensor("vt", [NKB, 128, NC_CHUNKS, 128], f16,
                          kind="ExternalInput").ap()
    wq_d = nc.dram_tensor("wq", [128, NC_CHUNKS, GCOLS], f16,
                          kind="ExternalInput").ap()
    wk_d = nc.dram_tensor("wk", [128, NC_CHUNKS, GCOLS], f16,
                          kind="ExternalInput").ap()
    wv_d = nc.dram_tensor("wv", [128, NC_CHUNKS, GCOLS], f16,
                          kind="ExternalInput").ap()
    wo_d = nc.dram_tensor("wo", [128, 4, D], f16, kind="ExternalInput").ap()
    yp_d = nc.dram_tensor("yp", [S, D], f16, kind="ExternalOutput").ap()

    with TileContext(nc) as tc:
        with tc.tile_pool(name="weights", bufs=1) as wpool, \
             tc.tile_pool(name="xt", bufs=1) as xtpool, \
             tc.tile_pool(name="vt", bufs=4) as vtpool, \
             tc.tile_pool(name="proj", bufs=1) as projpool, \
             tc.tile_pool(name="work", bufs=2) as workpool, \
             tc.tile_pool(name="psum", bufs=1, space="PSUM") as psp:

          for rep in range(reps):
            # ---- resident tiles ----
            wq_sb = wpool.tile([128, NC_CHUNKS, GCOLS], f16, name="wq_sb",
                               tag="wq")
            wk_sb = wpool.tile([128, NC_CHUNKS, GCOLS], f16, name="wk_sb",
                               tag="wk")
            wv_sb = wpool.tile([128, NC_CHUNKS, GCOLS], f16, name="wv_sb",
                               tag="wv")
            wo_sb = wpool.tile([128, 4, D], f16, name="wo_sb", tag="wo")
            kt_sb = xtpool.tile([128, NQB, NC_CHUNKS, QB], f16, name="kt_sb",
                                tag="kt")
            qt_sb = xtpool.tile([128, NQB, NC_CHUNKS, QB], f16, name="qt_sb",
                                tag="qt")
            # kT/qT: [dk-on-partitions, token]; chunk j holds head 2j on
            # partitions 0:64 and head 2j+1 on 64:128
            kT_sb = projpool.tile([128, 4, S], f16, name="kT_sb", tag="kT")
            qT_sb = projpool.tile([128, 4, S], f16, name="qT_sb", tag="qT")
            # v: [token-on-partitions, head, dim(+ones col at 64)]
            v_sb = projpool.tile([128, NKB, HPC, 66], f16, name="v_sb",
                                 tag="v")
            oT_sb = projpool.tile([128, 4, S], f16, name="oT_sb", tag="oT")

            # ---- PE warmup: dummy matmuls on scratch keep the PE busy
            # during the initial DMA wait so HAM un-throttles to 2.4 GHz
            # before real work arrives (and the cold ramp is not paid on it)
            scratch = workpool.tile([128, 640], f16, name="warm", tag="warm",
                                    bufs=1)
            nc.vector.memset(scratch[:], 0.5)
            wps = psp.tile([128, QB], f32, name="warm_ps", tag="pps", bufs=1)
            for w in range(18):
                nc.tensor.matmul(wps[:], scratch[:, 0:128],
                                 scratch[:, 128:640], start=True, stop=True)

            # ---- DMA emission (in consumption order; the 16 DMA engines
            # run these in parallel, so order mostly sets arrival priority)
            nc.sync.dma_start(out=wk_sb[:], in_=wk_d[:])
            nc.sync.dma_start(out=kt_sb[:, 0], in_=kt_d[0])
            nc.sync.dma_start(out=wq_sb[:], in_=wq_d[:])
            nc.sync.dma_start(out=qt_sb[:, 0], in_=qt_d[0])
            vt_tiles = {}

            def dma_vt(kb):
                t = vtpool.tile([128, NC_CHUNKS, 128], f16, name="vt_t",
                                tag="vtt")
                nc.sync.dma_start(out=t[:], in_=vt_d[kb])
                vt_tiles[kb] = t

            nc.sync.dma_start(out=wv_sb[:], in_=wv_d[:])
            dma_vt(0)
            dma_vt(1)

            dma_done = {"kt": 1, "qt": 1}

            def dma_x(which, n):
                sb, dr = (kt_sb, kt_d) if which == "kt" else (qt_sb, qt_d)
                nc.sync.dma_start(out=sb[:, n], in_=dr[n])
                dma_done[which] = n + 1

            # ---- projection building blocks ----
            def vproj_unit(kb):
                if kb >= 2:
                    dma_vt(kb)          # prefetch handled by pool bufs=4
                vt_t = vt_tiles[kb]
                nc.vector.memset(v_sb[:, kb, :, :], 1.0)
                ps = psp.tile([128, GCOLS], f32, name="vproj_ps", tag="pps",
                              bufs=1)
                for c in range(NC_CHUNKS):
                    nc.tensor.matmul(ps[:], vt_t[:, c, :], wv_sb[:, c, :],
                                     start=(c == 0), stop=(c == NC_CHUNKS - 1))
                nc.vector.tensor_copy(
                    v_sb[:, kb, :, 0:64],
                    ps[:].rearrange("p (h d) -> p h d", h=HPC))
                vt_tiles[kb] = None     # allow pool slot reuse

            def proj_half(w_sb, xt_sb, dst, m, n, half, holder):
                if half == 0:
                    holder[0] = psp.tile([128, QB], f32, name="proj_ps",
                                         tag="pps", bufs=1)
                ps = holder[0]
                for c in range(4 * half, 4 * half + 4):
                    nc.tensor.matmul(
                        ps[:], w_sb[:, c, m * 128:(m + 1) * 128],
                        xt_sb[:, n, c, :],
                        start=(c == 0), stop=(c == NC_CHUNKS - 1))
                if half == 1:
                    nc.vector.tensor_copy(dst[:, m, n * QB:(n + 1) * QB],
                                          ps[:])

            # ---- feed list: (deadline_iter, emit_fn) ----
            # iteration index = ((j*NQB)+qb)*NKB + kb over the attention loop
            def it_idx(j, qb, kb):
                return (j * NQB + qb) * NKB + kb

            feed = []

            def add_feed(deadline, fn):
                feed.append([deadline, fn])

            # v-projections: v[kb] needed at iter (0,0,kb); lookahead 2
            for kb in range(2, NKB):
                add_feed(it_idx(0, 0, kb) - 2, (lambda kb=kb: vproj_unit(kb)))
            # kT(m, n): needed by scores(j=m, qb=0, kb=4n); qT(m, qb) at
            # (m, qb, 0). Emitted as two half-units each, with targets
            # SPREAD across earlier iterations so the PE queue never gets a
            # burst of projection work in front of the score matmuls.
            units = []
            for m in range(4):
                for n in range(NQB):
                    if not (m == 0 and n == 0):
                        if m == 0:
                            tgt = max(1, 4 * (n - 1))
                        else:
                            # spread pair-m kT units over pair m-1 qb 1..2
                            tgt = it_idx(m - 1, 1, 0) + 6 * n
                        units.append(("kt", kT_sb, wk_sb, m, n,
                                      tgt, it_idx(m, 0, 4 * n) - 3))
                    if not (m == 0 and n == 0):
                        if n == 0:
                            tgt = it_idx(m - 1, 2, 8) + 4
                        else:
                            tgt = it_idx(m, n - 1, 6)
                        units.append(("qt", qT_sb, wq_sb, m, n,
                                      tgt, it_idx(m, n, 0) - 3))
            for which, dst, w_sb, m, n, tgt, dl in units:
                holder = [None]
                xt_sb = kt_sb if which == "kt" else qt_sb

                def mk(half, which=which, dst=dst, w_sb=w_sb, m=m, n=n,
                       xt_sb=xt_sb, holder=holder):
                    def fn():
                        if dma_done[which] <= n:
                            for nn in range(dma_done[which], n + 1):
                                dma_x(which, nn)
                        proj_half(w_sb, xt_sb, dst, m, n, half, holder)
                    return fn
                add_feed(min(tgt, dl - 1), mk(0))
                add_feed(min(tgt + 1, dl), mk(1))
            feed.sort(key=lambda e: e[0])
            # remaining input DMAs are pulled in by deadline; wo early on
            wo_loaded = [False]

            def load_wo():
                if not wo_loaded[0]:
                    nc.sync.dma_start(out=wo_sb[:], in_=wo_d[:])
                    wo_loaded[0] = True

            def pump(cur_iter, budget=1, horizon=16):
                # emit overdue units, plus up to `budget` units that come
                # due within `horizon` iterations (keeps filler spread out)
                while feed and feed[0][0] <= cur_iter:
                    feed.pop(0)[1]()
                while budget > 0 and feed and feed[0][0] <= cur_iter + horizon:
                    feed.pop(0)[1]()
                    budget -= 1
                if cur_iter >= it_idx(1, 2, 0):
                    load_wo()

            # ---- prefix projections ----
            h0 = [None]
            proj_half(wk_sb, kt_sb, kT_sb, 0, 0, 0, h0)
            proj_half(wk_sb, kt_sb, kT_sb, 0, 0, 1, h0)
            h1 = [None]
            proj_half(wq_sb, qt_sb, qT_sb, 0, 0, 0, h1)
            proj_half(wq_sb, qt_sb, qT_sb, 0, 0, 1, h1)
            vproj_unit(0)
            vproj_unit(1)

            # ---- output projection (per 128-token tile) ----
            def y_unit(t):
                # emitted inside DVE-exp iterations: the Act engine is
                # exp-idle there, so these copies don't delay the exp chain
                for n2 in range(2):
                    y_sb = workpool.tile([128, QB], f16, name="y_sb", tag="y",
                                         bufs=2)
                    ps = psp.tile([128, QB], f32, name="y_ps", tag="pps",
                                  bufs=1)
                    for c2 in range(4):
                        nc.tensor.matmul(
                            ps[:], oT_sb[:, c2, t * 128:(t + 1) * 128],
                            wo_sb[:, c2, n2 * QB:(n2 + 1) * QB],
                            start=(c2 == 0), stop=(c2 == 3))
                    nc.scalar.copy(y_sb[:], ps[:])
                    nc.sync.dma_start(
                        out=yp_d[t * 128:(t + 1) * 128,
                                 n2 * QB:(n2 + 1) * QB],
                        in_=y_sb[:])

            # ---- attention (software-pipelined: scores/exp run one
            #      iteration ahead of attn@V so the PE never waits on exp) --
            y_pending = []
            for j in range(4):
                h0i, h1i = 2 * j, 2 * j + 1
                unnorm0 = workpool.tile([64, NQB, QB], f16, name="unnorm0",
                                        tag="unnorm0", bufs=1)
                unnorm1 = workpool.tile([64, NQB, QB], f16, name="unnorm1",
                                        tag="unnorm1", bufs=1)
                deferred = [None]
                outs = {}
                prev = [None]

                def stage1(qb, kb, j=j):
                    sb2 = psp.tile([128, 2, QB], f32, name="sb2",
                                   tag="sbig", bufs=2)
                    nc.tensor.matmul(
                        sb2[:, 0, :],
                        kT_sb[0:64, j, kb * 128:(kb + 1) * 128],
                        qT_sb[0:64, j, qb * QB:(qb + 1) * QB],
                        start=True, stop=True)
                    nc.tensor.matmul(
                        sb2[:, 1, :],
                        kT_sb[64:128, j, kb * 128:(kb + 1) * 128],
                        qT_sb[64:128, j, qb * QB:(qb + 1) * QB],
                        start=True, stop=True)
                    pT = workpool.tile([128, 2, QB], f16, name="pT",
                                       tag="pT", bufs=4)
                    if kb in DVE_KB:
                        # Schraudolph 2^u: int16 bits of the fp16 result
                        nc.vector.tensor_scalar(
                            out=pT[:].rearrange("p a b -> p (a b)")
                                     .bitcast(i16),
                            in0=sb2[:].rearrange("p a b -> p (a b)"),
                            scalar1=1024.0, scalar2=SCHRAU_BIAS,
                            op0=MULT, op1=ADD)
                    else:
                        nc.scalar.activation(
                            pT[:].rearrange("p a b -> p (a b)"),
                            sb2[:].rearrange("p a b -> p (a b)"),
                            EXP, scale=LN2)
                    return pT

                def stage2(qb, kb, pT, j=j, h0i=h0i, h1i=h1i):
                    if kb == 0:
                        outs[qb] = (
                            psp.tile([128, QB], f32, name="out0", tag="out0",
                                     bufs=2),
                            psp.tile([128, QB], f32, name="out1", tag="out1",
                                     bufs=1))
                    out0, out1 = outs[qb]
                    nc.tensor.matmul(
                        out0[0:65, :], v_sb[:, kb, h0i, 0:65], pT[:, 0, :],
                        start=(kb == 0), stop=(kb == NKB - 1))
                    nc.tensor.matmul(
                        out1[0:65, :], v_sb[:, kb, h1i, 0:65], pT[:, 1, :],
                        start=(kb == 0), stop=(kb == NKB - 1))

                def qb_epilogue(qb, j=j):
                    # stage to SBUF fast (frees the PSUM accumulators);
                    # denominator rows go via the Act engine (it has slack),
                    # normalize is deferred one qb so copies never stall
                    out0, out1 = outs.pop(qb)
                    db = workpool.tile([1, 2, QB], f32, name="db", tag="db",
                                       bufs=2)
                    nc.vector.tensor_copy(db[:, 0, :], out0[64:65, :])
                    nc.vector.tensor_copy(db[:, 1, :], out1[64:65, :])
                    nc.vector.tensor_copy(unnorm0[:, qb, :], out0[0:64, :])
                    nc.vector.tensor_copy(unnorm1[:, qb, :], out1[0:64, :])

                    def _normalize(qb=qb, db=db, j=j):
                        rcp = workpool.tile([1, 2, QB], f32, name="rcp",
                                            tag="rcp", bufs=1)
                        if FAST_RECIP:
                            nc.vector.reciprocal_approx_fast(out=rcp[:],
                                                             in_=db[:])
                        else:
                            nc.vector.reciprocal(rcp[:], db[:])
                        rcph = workpool.tile([1, 2, QB], f16, name="rcph",
                                             tag="rcph", bufs=2)
                        nc.vector.tensor_copy(rcph[:], rcp[:])
                        rbc = workpool.tile([64, 2, QB], f16, name="rbc",
                                            tag="rbc", bufs=1)
                        nc.gpsimd.partition_broadcast(rbc[:, 0, :],
                                                      rcph[0:1, 0, :])
                        nc.gpsimd.partition_broadcast(rbc[:, 1, :],
                                                      rcph[0:1, 1, :])
                        nc.vector.tensor_mul(
                            oT_sb[0:64, j, qb * QB:(qb + 1) * QB],
                            unnorm0[0:64, qb, :], rbc[:, 0, :])
                        nc.vector.tensor_mul(
                            oT_sb[64:128, j, qb * QB:(qb + 1) * QB],
                            unnorm1[0:64, qb, :], rbc[:, 1, :])
                        if j == 3:
                            y_pending.extend(range(4 * qb, 4 * qb + 4))

                    if deferred[0] is not None:
                        deferred[0]()
                    deferred[0] = _normalize

                for qb in range(NQB):
                    for kb in range(NKB):
                        pump(it_idx(j, qb, kb))
                        pT = stage1(qb, kb)
                        if kb in DVE_KB and y_pending:
                            # Act engine is exp-idle this iteration
                            y_unit(y_pending.pop(0))
                        if prev[0] is not None:
                            pqb, pkb, ppT = prev[0]
                            stage2(pqb, pkb, ppT)
                            if pkb == NKB - 1:
                                qb_epilogue(pqb)
                        prev[0] = (qb, kb, pT)
                # flush the last iteration of this pair
                pqb, pkb, ppT = prev[0]
                stage2(pqb, pkb, ppT)
                qb_epilogue(pqb)
                deferred[0]()
                while j == 3 and feed:
                    feed.pop(0)[1]()
            while y_pending:
                y_unit(y_pending.pop(0))

    nc.compile()
    _prog_cache[key] = nc
    return nc


def _chunk_pT_nblk(x):
    """[S, D] -> [4, 128, 8, 512] fp16: out[n, p, c, t] = x[512n+t, 128c+p]."""
    return np.ascontiguousarray(
        x.reshape(NQB, QB, NC_CHUNKS, 128).transpose(0, 3, 2, 1))


def _chunk_pT_kb(x):
    """[S, D] -> [16, 128, 8, 128]: out[k, p, c, t] = x[128k+t, 128c+p]."""
    return np.ascontiguousarray(
        x.reshape(NKB, 128, NC_CHUNKS, 128).transpose(0, 3, 2, 1))


def _chunk_w(w):
    """[D, GCOLS] -> [128, 8, GCOLS]: out[p, c, m] = w[128c+p, m]."""
    return np.ascontiguousarray(
        w.reshape(NC_CHUNKS, 128, w.shape[1]).transpose(1, 0, 2))


def prepare_in_maps(Q, K, V, W_Q, W_K, W_V, W_O):
    f16 = np.float16
    wq_scaled = (W_Q.astype(np.float32) * np.float32(LOG2E / 8.0))
    qt = [_chunk_pT_nblk(Q[b].astype(f16)) for b in range(B)]
    kt = [_chunk_pT_nblk(K[b].astype(f16)) for b in range(B)]
    vt = [_chunk_pT_kb(V[b].astype(f16)) for b in range(B)]
    wq = [_chunk_w(wq_scaled[:, g * GCOLS:(g + 1) * GCOLS].astype(f16))
          for g in range(2)]
    wk = [_chunk_w(W_K[:, g * GCOLS:(g + 1) * GCOLS].astype(f16))
          for g in range(2)]
    wv = [_chunk_w(W_V[:, g * GCOLS:(g + 1) * GCOLS].astype(f16))
          for g in range(2)]
    wo = [np.ascontiguousarray(
        W_O[g * GCOLS:(g + 1) * GCOLS, :].astype(f16)
        .reshape(4, 128, D).transpose(1, 0, 2)) for g in range(2)]
    in_maps = []
    for c in range(NCORES):
        b, g = c // 2, c % 2
        in_maps.append({
            "qt": qt[b], "kt": kt[b], "vt": vt[b],
            "wq": wq[g], "wk": wk[g], "wv": wv[g], "wo": wo[g],
        })
    return in_maps


def execute(nc, in_maps):
    from concourse.bass_utils import run_bass_kernel_spmd
    res = run_bass_kernel_spmd(nc, in_maps, list(range(NCORES)))
    return res


def _numpy_fallback(Q, K, V, mask, W_Q, W_K, W_V, W_O):
    B_, S1, _ = Q.shape
    q = (Q.reshape(-1, D) @ W_Q).reshape(B_, S1, H, DK).transpose(0, 2, 1, 3)
    k = (K.reshape(-1, D) @ W_K).reshape(B_, S1, H, DK).transpose(0, 2, 1, 3)
    v = (V.reshape(-1, D) @ W_V).reshape(B_, S1, H, DK).transpose(0, 2, 1, 3)
    out = np.empty((B_, H, S1, DK), np.float32)
    for b in range(B_):
        for h in range(H):
            s = (q[b, h] @ k[b, h].T) / math.sqrt(DK)
            s = np.where(mask[b] == 0, np.float32(-1e9), s)
            s = s - s.max(axis=-1, keepdims=True)
            e = np.exp(s)
            p = e / e.sum(axis=-1, keepdims=True)
            out[b, h] = p @ v[b, h]
    o = out.transpose(0, 2, 1, 3).reshape(B_, S1, D)
    return (o.reshape(-1, D) @ W_O).reshape(B_, S1, D).astype(np.float32)


def kernel(Q, K, V, mask, W_Q, W_K, W_V, W_O):
    Q = np.asarray(Q); K = np.asarray(K); V = np.asarray(V)
    mask = np.asarray(mask)
    W_Q = np.asarray(W_Q); W_K = np.asarray(W_K)
    W_V = np.asarray(W_V); W_O = np.asarray(W_O)
    if (mask == 0).any():
        # spec guarantees an all-ones mask; this path is correctness insurance
        return _numpy_fallback(Q, K, V, mask, W_Q, W_K, W_V, W_O)
    nc = build_program()
    in_maps = prepare_in_maps(Q, K, V, W_Q, W_K, W_V, W_O)
    res = execute(nc, in_maps)
    out = np.empty((B, S, D), np.float32)
    for b in range(B):
        out[b] = (res.results[2 * b]["yp"].astype(np.float32)
                  + res.results[2 * b + 1]["yp"].astype(np.float32))
    return out


# revision 32
# speedup vs baseline: 1.0512x; 1.0512x over previous
"""Multi-head attention (B=4, S=2048, d_model=1024, H=16) on 8 TRN2 NeuronCores.

Sharding: tensor-parallel over heads x data-parallel over batch.
Core c handles batch b=c//2 and head group g=c%2 (8 heads = 512 of the
1024 d_model columns of W_Q/W_K/W_V, and 512 rows of W_O). Each core
produces a partial output Y_partial[b] = O_g @ W_O[g-rows, :]; the host
sums the two partials per batch.

Device-side dataflow per core (all matmul operands fp16, accum fp32):
  - log2e/8 is folded into W_Q on the host, so scores arrive in the
    log2 domain: exp(s/8) == 2^u with u the raw matmul output
  - k^T, q^T = W^T X^T         (lhsT = W chunk, rhs = X^T chunk)
  - v = X @ W_V   in [token, head-dim] layout, with a ones column
  - per head pair, per 128-ktok block: scores^T = k^T.T q^T -> PSUM
    (row-tiled 64x128 pair, concurrent in the PE array)
    2^u -> P^T fp16 via Act-engine exp (scale=ln2) for 3 of 4 blocks,
    and via a single DVE tensor_scalar Schraudolph (int16 bit trick)
    for the 4th -- splits the exp load across both engines
    out^T_ext += [v_h | 1].T @ P^T   (row 64 = softmax denominator)
  - out^T / denominator -> O^T (reciprocal_approx_fast + gpsimd bcast)
  - Y_partial = O @ W_O slice -> DRAM fp16, summed on host

Scheduling: projections are emitted with per-iteration deadlines into
the attention stream (earliest exp at ~13us instead of ~70us), and the
output projection is emitted eagerly inside the last pair's qb loop.
"""

import math
import numpy as np

B = 4
S = 2048
D = 1024
H = 16
DK = 64
NCORES = 8
HPC = 8          # heads per core
GCOLS = 512      # d_model columns per head group
QB = 512         # q-token block (PSUM bank free dim)
NQB = S // QB    # 4
NKB = S // 128   # 16 k-token blocks
NC_CHUNKS = D // 128  # 8 contraction chunks

LOG2E = math.log2(math.e)
LN2 = math.log(2.0)
# fp16 Schraudolph: j = round(1024*u + (15*1024 - C)); bits(j) ~ 2^u
SCHRAU_BIAS = float(15 * 1024 - 60)
# which kb iterations run exp on DVE instead of Act (1 of 4 = 25%)
import os
DVE_KB = (frozenset((3, 7, 11, 15)) if os.environ.get("NO_DVE_EXP") != "1"
          else frozenset())
FAST_RECIP = os.environ.get("NO_FAST_RECIP") != "1"

_prog_cache = {}


def build_program(reps=1):
    key = (reps,)
    if key in _prog_cache:
        return _prog_cache[key]

    import concourse.bacc as bacc
    import concourse.mybir as mybir
    from concourse.tile import TileContext

    f16 = mybir.dt.float16
    i16 = mybir.dt.int16
    f32 = mybir.dt.float32
    EXP = mybir.ActivationFunctionType.Exp
    MULT = mybir.AluOpType.mult
    ADD = mybir.AluOpType.add

    nc = bacc.Bacc("TRN2", target_bir_lowering=False, debug=False,
                   num_devices=NCORES)

    # DRAM parameters (per-core shards, pre-laid-out on host)
    # token-block-major for kt/qt, kb-major for vt => in-order small DMAs
    qt_d = nc.dram_tensor("qt", [NQB, 128, NC_CHUNKS, QB], f16,
                          kind="ExternalInput").ap()
    kt_d = nc.dram_tensor("kt", [NQB, 128, NC_CHUNKS, QB], f16,
                          kind="ExternalInput").ap()
    vt_d = nc.dram_tensor("vt", [NKB, 128, NC_CHUNKS, 128], f16,
                          kind="ExternalInput").ap()
    wq_d = nc.dram_tensor("wq", [128, NC_CHUNKS, GCOLS], f16,
                          kind="ExternalInput").ap()
    wk_d = nc.dram_tensor("wk", [128, NC_CHUNKS, GCOLS], f16,
                          kind="ExternalInput").ap()
    wv_d = nc.dram_tensor("wv", [128, NC_CHUNKS, GCOLS], f16,
                          kind="ExternalInput").ap()
    wo_d = nc.dram_tensor("wo", [128, 4, D], f16, kind="ExternalInput").ap()
    yp_d = nc.dram_tensor("yp", [S, D], f16, kind="ExternalOutput").ap()

    with TileContext(nc) as tc:
        with tc.tile_pool(name="weights", bufs=1) as wpool, \
             tc.tile_pool(name="xt", bufs=1) as xtpool, \
             tc.tile_pool(name="vt", bufs=4) as vtpool, \
             tc.tile_pool(name="proj", bufs=1) as projpool, \
             tc.tile_pool(name="work", bufs=2) as workpool, \
             tc.tile_pool(name="psum", bufs=1, space="PSUM") as psp:

          for rep in range(reps):
            # ---- resident tiles ----
            wq_sb = wpool.tile([128, NC_CHUNKS, GCOLS], f16, name="wq_sb",
                               tag="wq")
            wk_sb = wpool.tile([128, NC_CHUNKS, GCOLS], f16, name="wk_sb",
                               tag="wk")
            wv_sb = wpool.tile([128, NC_CHUNKS, GCOLS], f16, name="wv_sb",
                               tag="wv")
            wo_sb = wpool.tile([128, 4, D], f16, name="wo_sb", tag="wo")
            kt_sb = xtpool.tile([128, NQB, NC_CHUNKS, QB], f16, name="kt_sb",
                                tag="kt")
            qt_sb = xtpool.tile([128, NQB, NC_CHUNKS, QB], f16, name="qt_sb",
                                tag="qt")
            # kT/qT: [dk-on-partitions, token]; chunk j holds head 2j on
            # partitions 0:64 and head 2j+1 on 64:128
            kT_sb = projpool.tile([128, 4, S], f16, name="kT_sb", tag="kT")
            qT_sb = projpool.tile([128, 4, S], f16, name="qT_sb", tag="qT")
            # v: [token-on-partitions, head, dim(+ones col at 64)]
            v_sb = projpool.tile([128, NKB, HPC, 66], f16, name="v_sb",
                                 tag="v")
            oT_sb = projpool.tile([128, 4, S], f16, name="oT_sb", tag="oT")

            # ---- PE warmup: dummy matmuls on scratch keep the PE busy
            # during the initial DMA wait so HAM un-throttles to 2.4 GHz
            # before real work arrives (and the cold ramp is not paid on it)
            scratch = workpool.tile([128, 640], f16, name="warm", tag="warm",
                                    bufs=1)
            nc.vector.memset(scratch[:], 0.5)
            wps = psp.tile([128, QB], f32, name="warm_ps", tag="pps", bufs=2)
            for w in range(18):
                nc.tensor.matmul(wps[:], scratch[:, 0:128],
                                 scratch[:, 128:640], start=True, stop=True)

            # ---- DMA emission (in consumption order; the 16 DMA engines
            # run these in parallel, so order mostly sets arrival priority)
            nc.sync.dma_start(out=wk_sb[:], in_=wk_d[:])
            nc.sync.dma_start(out=kt_sb[:, 0], in_=kt_d[0])
            nc.sync.dma_start(out=wq_sb[:], in_=wq_d[:])
            nc.sync.dma_start(out=qt_sb[:, 0], in_=qt_d[0])
            vt_tiles = {}

            def dma_vt(kb):
                t = vtpool.tile([128, NC_CHUNKS, 128], f16, name="vt_t",
                                tag="vtt")
                nc.sync.dma_start(out=t[:], in_=vt_d[kb])
                vt_tiles[kb] = t

            nc.sync.dma_start(out=wv_sb[:], in_=wv_d[:])
            dma_vt(0)
            dma_vt(1)

            dma_done = {"kt": 1, "qt": 1}

            def dma_x(which, n):
                sb, dr = (kt_sb, kt_d) if which == "kt" else (qt_sb, qt_d)
                nc.sync.dma_start(out=sb[:, n], in_=dr[n])
                dma_done[which] = n + 1

            # ---- projection building blocks ----
            def vproj_unit(kb):
                if kb >= 2:
                    dma_vt(kb)          # prefetch handled by pool bufs=4
                vt_t = vt_tiles[kb]
                nc.vector.memset(v_sb[:, kb, :, :], 1.0)
                ps = psp.tile([128, GCOLS], f32, name="vproj_ps", tag="pps",
                              bufs=2)
                for c in range(NC_CHUNKS):
                    nc.tensor.matmul(ps[:], vt_t[:, c, :], wv_sb[:, c, :],
                                     start=(c == 0), stop=(c == NC_CHUNKS - 1))
                nc.vector.tensor_copy(
                    v_sb[:, kb, :, 0:64],
                    ps[:].rearrange("p (h d) -> p h d", h=HPC))
                vt_tiles[kb] = None     # allow pool slot reuse

            def proj_half(w_sb, xt_sb, dst, m, n, half, holder):
                if half == 0:
                    holder[0] = psp.tile([128, QB], f32, name="proj_ps",
                                         tag="pps", bufs=2)
                ps = holder[0]
                for c in range(4 * half, 4 * half + 4):
                    nc.tensor.matmul(
                        ps[:], w_sb[:, c, m * 128:(m + 1) * 128],
                        xt_sb[:, n, c, :],
                        start=(c == 0), stop=(c == NC_CHUNKS - 1))
                if half == 1:
                    nc.vector.tensor_copy(dst[:, m, n * QB:(n + 1) * QB],
                                          ps[:])

            # ---- feed list: (deadline_iter, emit_fn) ----
            # iteration index = ((j*NQB)+qb)*NKB + kb over the attention loop
            def it_idx(j, qb, kb):
                return (j * NQB + qb) * NKB + kb

            feed = []

            def add_feed(deadline, fn):
                feed.append([deadline, fn])

            # v-projections: v[kb] needed at iter (0,0,kb); lookahead 2
            for kb in range(2, NKB):
                add_feed(it_idx(0, 0, kb) - 2, (lambda kb=kb: vproj_unit(kb)))
            # kT(m, n): needed by scores(j=m, qb=0, kb=4n); qT(m, qb) at
            # (m, qb, 0). Emitted as two half-units each, with targets
            # SPREAD across earlier iterations so the PE queue never gets a
            # burst of projection work in front of the score matmuls.
            units = []
            for m in range(4):
                for n in range(NQB):
                    if not (m == 0 and n == 0):
                        if m == 0:
                            tgt = max(1, 4 * (n - 1))
                        else:
                            # spread pair-m kT units over pair m-1 qb 1..2
                            tgt = it_idx(m - 1, 1, 0) + 6 * n
                        units.append(("kt", kT_sb, wk_sb, m, n,
                                      tgt, it_idx(m, 0, 4 * n) - 3))
                    if not (m == 0 and n == 0):
                        if n == 0:
                            tgt = it_idx(m - 1, 2, 8) + 4
                        else:
                            tgt = it_idx(m, n - 1, 6)
                        units.append(("qt", qT_sb, wq_sb, m, n,
                                      tgt, it_idx(m, n, 0) - 3))
            for which, dst, w_sb, m, n, tgt, dl in units:
                holder = [None]
                xt_sb = kt_sb if which == "kt" else qt_sb

                def mk(half, which=which, dst=dst, w_sb=w_sb, m=m, n=n,
                       xt_sb=xt_sb, holder=holder):
                    def fn():
                        if dma_done[which] <= n:
                            for nn in range(dma_done[which], n + 1):
                                dma_x(which, nn)
                        proj_half(w_sb, xt_sb, dst, m, n, half, holder)
                    return fn
                add_feed(min(tgt, dl - 1), mk(0))
                add_feed(min(tgt + 1, dl), mk(1))
            feed.sort(key=lambda e: e[0])
            # remaining input DMAs are pulled in by deadline; wo early on
            wo_loaded = [False]

            def load_wo():
                if not wo_loaded[0]:
                    nc.sync.dma_start(out=wo_sb[:], in_=wo_d[:])
                    wo_loaded[0] = True

            def pump(cur_iter, budget=1, horizon=16):
                # emit overdue units, plus up to `budget` units that come
                # due within `horizon` iterations (keeps filler spread out)
                while feed and feed[0][0] <= cur_iter:
                    feed.pop(0)[1]()
                while budget > 0 and feed and feed[0][0] <= cur_iter + horizon:
                    feed.pop(0)[1]()
                    budget -= 1
                if cur_iter >= it_idx(1, 2, 0):
                    load_wo()

            # ---- prefix projections ----
            h0 = [None]
            proj_half(wk_sb, kt_sb, kT_sb, 0, 0, 0, h0)
            proj_half(wk_sb, kt_sb, kT_sb, 0, 0, 1, h0)
            h1 = [None]
            proj_half(wq_sb, qt_sb, qT_sb, 0, 0, 0, h1)
            proj_half(wq_sb, qt_sb, qT_sb, 0, 0, 1, h1)
            vproj_unit(0)
            vproj_unit(1)

            # ---- output projection (per 128-token tile) ----
            def y_unit(t):
                # emitted inside DVE-exp iterations: the Act engine is
                # exp-idle there, so these copies don't delay the exp chain
                for n2 in range(2):
                    y_sb = workpool.tile([128, QB], f16, name="y_sb", tag="y",
                                         bufs=2)
                    ps = psp.tile([128, QB], f32, name="y_ps", tag="pps",
                                  bufs=2)
                    for c2 in range(4):
                        nc.tensor.matmul(
                            ps[:], oT_sb[:, c2, t * 128:(t + 1) * 128],
                            wo_sb[:, c2, n2 * QB:(n2 + 1) * QB],
                            start=(c2 == 0), stop=(c2 == 3))
                    nc.scalar.copy(y_sb[:], ps[:])
                    nc.sync.dma_start(
                        out=yp_d[t * 128:(t + 1) * 128,
                                 n2 * QB:(n2 + 1) * QB],
                        in_=y_sb[:])

            # ---- attention (software-pipelined: scores/exp run one
            #      iteration ahead of attn@V so the PE never waits on exp) --
            y_pending = []
            for j in range(4):
                h0i, h1i = 2 * j, 2 * j + 1
                unnorm0 = workpool.tile([64, NQB, QB], f16, name="unnorm0",
                                        tag="unnorm0", bufs=1)
                unnorm1 = workpool.tile([64, NQB, QB], f16, name="unnorm1",
                                        tag="unnorm1", bufs=1)
                deferred = [None]
                outs = {}
                prev = [None]

                def stage1(qb, kb, j=j):
                    sb2 = psp.tile([128, 2, QB], f32, name="sb2",
                                   tag="sbig", bufs=2)
                    nc.tensor.matmul(
                        sb2[:, 0, :],
                        kT_sb[0:64, j, kb * 128:(kb + 1) * 128],
                        qT_sb[0:64, j, qb * QB:(qb + 1) * QB],
                        start=True, stop=True)
                    nc.tensor.matmul(
                        sb2[:, 1, :],
                        kT_sb[64:128, j, kb * 128:(kb + 1) * 128],
                        qT_sb[64:128, j, qb * QB:(qb + 1) * QB],
                        start=True, stop=True)
                    pT = workpool.tile([128, 2, QB], f16, name="pT",
                                       tag="pT", bufs=4)
                    if kb in DVE_KB:
                        # Schraudolph 2^u: int16 bits of the fp16 result
                        nc.vector.tensor_scalar(
                            out=pT[:].rearrange("p a b -> p (a b)")
                                     .bitcast(i16),
                            in0=sb2[:].rearrange("p a b -> p (a b)"),
                            scalar1=1024.0, scalar2=SCHRAU_BIAS,
                            op0=MULT, op1=ADD)
                    else:
                        nc.scalar.activation(
                            pT[:].rearrange("p a b -> p (a b)"),
                            sb2[:].rearrange("p a b -> p (a b)"),
                            EXP, scale=LN2)
                    return pT

                def stage2(qb, kb, pT, j=j, h0i=h0i, h1i=h1i):
                    if kb == 0:
                        outs[qb] = (
                            psp.tile([128, QB], f32, name="out0", tag="out0",
                                     bufs=1),
                            psp.tile([128, QB], f32, name="out1", tag="out1",
                                     bufs=1))
                    out0, out1 = outs[qb]
                    nc.tensor.matmul(
                        out0[0:65, :], v_sb[:, kb, h0i, 0:65], pT[:, 0, :],
                        start=(kb == 0), stop=(kb == NKB - 1))
                    nc.tensor.matmul(
                        out1[0:65, :], v_sb[:, kb, h1i, 0:65], pT[:, 1, :],
                        start=(kb == 0), stop=(kb == NKB - 1))

                def qb_epilogue(qb, j=j):
                    # stage to SBUF fast (frees the PSUM accumulators);
                    # denominator rows go via the Act engine (it has slack),
                    # normalize is deferred one qb so copies never stall
                    out0, out1 = outs.pop(qb)
                    db = workpool.tile([1, 2, QB], f32, name="db", tag="db",
                                       bufs=2)
                    nc.vector.tensor_copy(db[:, 0, :], out0[64:65, :])
                    nc.vector.tensor_copy(db[:, 1, :], out1[64:65, :])
                    nc.vector.tensor_copy(unnorm0[:, qb, :], out0[0:64, :])
                    nc.vector.tensor_copy(unnorm1[:, qb, :], out1[0:64, :])

                    def _normalize(qb=qb, db=db, j=j):
                        rcp = workpool.tile([1, 2, QB], f32, name="rcp",
                                            tag="rcp", bufs=1)
                        if FAST_RECIP:
                            nc.vector.reciprocal_approx_fast(out=rcp[:],
                                                             in_=db[:])
                        else:
                            nc.vector.reciprocal(rcp[:], db[:])
                        rcph = workpool.tile([1, 2, QB], f16, name="rcph",
                                             tag="rcph", bufs=2)
                        nc.vector.tensor_copy(rcph[:], rcp[:])
                        rbc = workpool.tile([64, 2, QB], f16, name="rbc",
                                            tag="rbc", bufs=1)
                        nc.gpsimd.partition_broadcast(rbc[:, 0, :],
                                                      rcph[0:1, 0, :])
                        nc.gpsimd.partition_broadcast(rbc[:, 1, :],
                                                      rcph[0:1, 1, :])
                        nc.vector.tensor_mul(
                            oT_sb[0:64, j, qb * QB:(qb + 1) * QB],
                            unnorm0[0:64, qb, :], rbc[:, 0, :])
                        nc.vector.tensor_mul(
                            oT_sb[64:128, j, qb * QB:(qb + 1) * QB],
                            unnorm1[0:64, qb, :], rbc[:, 1, :])
                        if j == 3:
                            y_pending.extend(range(4 * qb, 4 * qb + 4))

                    if deferred[0] is not None:
                        deferred[0]()
                    deferred[0] = _normalize

                for qb in range(NQB):
                    for kb in range(NKB):
                        pump(it_idx(j, qb, kb))
                        pT = stage1(qb, kb)
                        if kb in DVE_KB and y_pending:
                            # Act engine is exp-idle this iteration
                            y_unit(y_pending.pop(0))
                        if prev[0] is not None:
                            pqb, pkb, ppT = prev[0]
                            stage2(pqb, pkb, ppT)
                            if pkb == NKB - 1:
                                qb_epilogue(pqb)
                        prev[0] = (qb, kb, pT)
                # flush the last iteration of this pair
                pqb, pkb, ppT = prev[0]
                stage2(pqb, pkb, ppT)
                qb_epilogue(pqb)
                deferred[0]()
                while j == 3 and feed:
                    feed.pop(0)[1]()
            while y_pending:
                y_unit(y_pending.pop(0))

    nc.compile()
    _prog_cache[key] = nc
    return nc


def _chunk_pT_nblk(x):
    """[S, D] -> [4, 128, 8, 512] fp16: out[n, p, c, t] = x[512n+t, 128c+p]."""
    return np.ascontiguousarray(
        x.reshape(NQB, QB, NC_CHUNKS, 128).transpose(0, 3, 2, 1))


def _chunk_pT_kb(x):
    """[S, D] -> [16, 128, 8, 128]: out[k, p, c, t] = x[128k+t, 128c+p]."""
    return np.ascontiguousarray(
        x.reshape(NKB, 128, NC_CHUNKS, 128).transpose(0, 3, 2, 1))


def _chunk_w(w):
    """[D, GCOLS] -> [128, 8, GCOLS]: out[p, c, m] = w[128c+p, m]."""
    return np.ascontiguousarray(
        w.reshape(NC_CHUNKS, 128, w.shape[1]).transpose(1, 0, 2))


def prepare_in_maps(Q, K, V, W_Q, W_K, W_V, W_O):
    f16 = np.float16
    wq_scaled = (W_Q.astype(np.float32) * np.float32(LOG2E / 8.0))
    qt = [_chunk_pT_nblk(Q[b].astype(f16)) for b in range(B)]
    kt = [_chunk_pT_nblk(K[b].astype(f16)) for b in range(B)]
    vt = [_chunk_pT_kb(V[b].astype(f16)) for b in range(B)]
    wq = [_chunk_w(wq_scaled[:, g * GCOLS:(g + 1) * GCOLS].astype(f16))
          for g in range(2)]
    wk = [_chunk_w(W_K[:, g * GCOLS:(g + 1) * GCOLS].astype(f16))
          for g in range(2)]
    wv = [_chunk_w(W_V[:, g * GCOLS:(g + 1) * GCOLS].astype(f16))
          for g in range(2)]
    wo = [np.ascontiguousarray(
        W_O[g * GCOLS:(g + 1) * GCOLS, :].astype(f16)
        .reshape(4, 128, D).transpose(1, 0, 2)) for g in range(2)]
    in_maps = []
    for c in range(NCORES):
        b, g = c // 2, c % 2
        in_maps.append({
            "qt": qt[b], "kt": kt[b], "vt": vt[b],
            "wq": wq[g], "wk": wk[g], "wv": wv[g], "wo": wo[g],
        })
    return in_maps


def execute(nc, in_maps):
    from concourse.bass_utils import run_bass_kernel_spmd
    res = run_bass_kernel_spmd(nc, in_maps, list(range(NCORES)))
    return res


def _numpy_fallback(Q, K, V, mask, W_Q, W_K, W_V, W_O):
    B_, S1, _ = Q.shape
    q = (Q.reshape(-1, D) @ W_Q).reshape(B_, S1, H, DK).transpose(0, 2, 1, 3)
    k = (K.reshape(-1, D) @ W_K).reshape(B_, S1, H, DK).transpose(0, 2, 1, 3)
    v = (V.reshape(-1, D) @ W_V).reshape(B_, S1, H, DK).transpose(0, 2, 1, 3)
    out = np.empty((B_, H, S1, DK), np.float32)
    for b in range(B_):
        for h in range(H):
            s = (q[b, h] @ k[b, h].T) / math.sqrt(DK)
            s = np.where(mask[b] == 0, np.float32(-1e9), s)
            s = s - s.max(axis=-1, keepdims=True)
            e = np.exp(s)
            p = e / e.sum(axis=-1, keepdims=True)
            out[b, h] = p @ v[b, h]
    o = out.transpose(0, 2, 1, 3).reshape(B_, S1, D)
    return (o.reshape(-1, D) @ W_O).reshape(B_, S1, D).astype(np.float32)


def kernel(Q, K, V, mask, W_Q, W_K, W_V, W_O):
    Q = np.asarray(Q); K = np.asarray(K); V = np.asarray(V)
    mask = np.asarray(mask)
    W_Q = np.asarray(W_Q); W_K = np.asarray(W_K)
    W_V = np.asarray(W_V); W_O = np.asarray(W_O)
    if (mask == 0).any():
        # spec guarantees an all-ones mask; this path is correctness insurance
        return _numpy_fallback(Q, K, V, mask, W_Q, W_K, W_V, W_O)
    nc = build_program()
    in_maps = prepare_in_maps(Q, K, V, W_Q, W_K, W_V, W_O)
    res = execute(nc, in_maps)
    out = np.empty((B, S, D), np.float32)
    for b in range(B):
        out[b] = (res.results[2 * b]["yp"].astype(np.float32)
                  + res.results[2 * b + 1]["yp"].astype(np.float32))
    return out


# revision 36
# speedup vs baseline: 1.1137x; 1.0595x over previous
"""Multi-head attention (B=4, S=2048, d_model=1024, H=16) on 8 TRN2 NeuronCores.

Sharding: tensor-parallel over heads x data-parallel over batch.
Core c handles batch b=c//2 and head group g=c%2 (8 heads = 512 of the
1024 d_model columns of W_Q/W_K/W_V, and 512 rows of W_O). Each core
produces a partial output Y_partial[b] = O_g @ W_O[g-rows, :]; the host
sums the two partials per batch.

Device-side dataflow per core (all matmul operands fp16, accum fp32):
  - log2e/8 is folded into W_Q on the host, so scores arrive in the
    log2 domain: exp(s/8) == 2^u with u the raw matmul output
  - k^T, q^T = W^T X^T         (lhsT = W chunk, rhs = X^T chunk)
  - v = X @ W_V   in [token, head-dim] layout, with a ones column
  - per head pair, per 128-ktok block: scores^T = k^T.T q^T -> PSUM
    (row-tiled 64x128 pair, concurrent in the PE array)
    2^u -> P^T fp16 via Act-engine exp (scale=ln2) for 3 of 4 blocks,
    and via a single DVE tensor_scalar Schraudolph (int16 bit trick)
    for the 4th -- splits the exp load across both engines
    out^T_ext += [v_h | 1].T @ P^T   (row 64 = softmax denominator)
  - out^T / denominator -> O^T (reciprocal_approx_fast + gpsimd bcast)
  - Y_partial = O @ W_O slice -> DRAM fp16, summed on host

Scheduling: projections are emitted with per-iteration deadlines into
the attention stream (earliest exp at ~13us instead of ~70us), and the
output projection is emitted eagerly inside the last pair's qb loop.
"""

import math
import numpy as np

B = 4
S = 2048
D = 1024
H = 16
DK = 64
NCORES = 8
HPC = 8          # heads per core
GCOLS = 512      # d_model columns per head group
QB = 512         # q-token block (PSUM bank free dim)
NQB = S // QB    # 4
NKB = S // 128   # 16 k-token blocks
NC_CHUNKS = D // 128  # 8 contraction chunks

LOG2E = math.log2(math.e)
LN2 = math.log(2.0)
# fp16 Schraudolph: j = round(1024*u + (15*1024 - C)); bits(j) ~ 2^u
SCHRAU_BIAS = float(15 * 1024 - 60)
# which kb iterations run exp on DVE instead of Act (1 of 4 = 25%)
import os
DVE_KB = (frozenset((3, 7, 11, 15)) if os.environ.get("NO_DVE_EXP") != "1"
          else frozenset())
FAST_RECIP = os.environ.get("NO_FAST_RECIP") != "1"

_prog_cache = {}


def build_program(reps=1):
    key = (reps,)
    if key in _prog_cache:
        return _prog_cache[key]

    import concourse.bacc as bacc
    import concourse.mybir as mybir
    from concourse.tile import TileContext

    f16 = mybir.dt.float16
    i16 = mybir.dt.int16
    f32 = mybir.dt.float32
    EXP = mybir.ActivationFunctionType.Exp
    MULT = mybir.AluOpType.mult
    ADD = mybir.AluOpType.add

    nc = bacc.Bacc("TRN2", target_bir_lowering=False, debug=False,
                   num_devices=NCORES)

    # DRAM parameters (per-core shards, pre-laid-out on host)
    # token-block-major for kt/qt, kb-major for vt => in-order small DMAs
    qt_d = nc.dram_tensor("qt", [NQB, 128, NC_CHUNKS, QB], f16,
                          kind="ExternalInput").ap()
    kt_d = nc.dram_tensor("kt", [NQB, 128, NC_CHUNKS, QB], f16,
                          kind="ExternalInput").ap()
    vt_d = nc.dram_tensor("vt", [NKB, 128, NC_CHUNKS, 128], f16,
                          kind="ExternalInput").ap()
    wq_d = nc.dram_tensor("wq", [128, NC_CHUNKS, GCOLS], f16,
                          kind="ExternalInput").ap()
    wk_d = nc.dram_tensor("wk", [128, NC_CHUNKS, GCOLS], f16,
                          kind="ExternalInput").ap()
    wv_d = nc.dram_tensor("wv", [128, NC_CHUNKS, GCOLS], f16,
                          kind="ExternalInput").ap()
    wo_d = nc.dram_tensor("wo", [128, 4, D], f16, kind="ExternalInput").ap()
    yp_d = nc.dram_tensor("yp", [S, D], f16, kind="ExternalOutput").ap()

    with TileContext(nc) as tc:
        with tc.tile_pool(name="weights", bufs=1) as wpool, \
             tc.tile_pool(name="xt", bufs=1) as xtpool, \
             tc.tile_pool(name="vt", bufs=4) as vtpool, \
             tc.tile_pool(name="proj", bufs=1) as projpool, \
             tc.tile_pool(name="work", bufs=2) as workpool, \
             tc.tile_pool(name="psum", bufs=1, space="PSUM") as psp:

          for rep in range(reps):
            # ---- resident tiles ----
            wq_sb = wpool.tile([128, NC_CHUNKS, GCOLS], f16, name="wq_sb",
                               tag="wq")
            wk_sb = wpool.tile([128, NC_CHUNKS, GCOLS], f16, name="wk_sb",
                               tag="wk")
            wv_sb = wpool.tile([128, NC_CHUNKS, GCOLS], f16, name="wv_sb",
                               tag="wv")
            wo_sb = wpool.tile([128, 4, D], f16, name="wo_sb", tag="wo")
            kt_sb = xtpool.tile([128, NQB, NC_CHUNKS, QB], f16, name="kt_sb",
                                tag="kt")
            qt_sb = xtpool.tile([128, NQB, NC_CHUNKS, QB], f16, name="qt_sb",
                                tag="qt")
            # kT/qT: [dk-on-partitions, token]; chunk j holds head 2j on
            # partitions 0:64 and head 2j+1 on 64:128
            kT_sb = projpool.tile([128, 4, S], f16, name="kT_sb", tag="kT")
            qT_sb = projpool.tile([128, 4, S], f16, name="qT_sb", tag="qT")
            # v: [token-on-partitions, head, dim(+ones col at 64)]
            v_sb = projpool.tile([128, NKB, HPC, 66], f16, name="v_sb",
                                 tag="v")
            oT_sb = projpool.tile([128, 4, S], f16, name="oT_sb", tag="oT")

            # ---- PE warmup: dummy matmuls on scratch keep the PE busy
            # during the initial DMA wait so HAM un-throttles to 2.4 GHz
            # before real work arrives (and the cold ramp is not paid on it)
            scratch = workpool.tile([128, 640], f16, name="warm", tag="warm",
                                    bufs=1)
            nc.vector.memset(scratch[:], 0.5)
            wps = psp.tile([128, QB], f32, name="warm_ps", tag="pps", bufs=2)
            for w in range(18):
                nc.tensor.matmul(wps[:], scratch[:, 0:128],
                                 scratch[:, 128:640], start=True, stop=True)

            # ---- DMA emission (in consumption order; the 16 DMA engines
            # run these in parallel, so order mostly sets arrival priority)
            nc.sync.dma_start(out=wk_sb[:], in_=wk_d[:])
            nc.sync.dma_start(out=kt_sb[:, 0], in_=kt_d[0])
            nc.sync.dma_start(out=wq_sb[:], in_=wq_d[:])
            nc.sync.dma_start(out=qt_sb[:, 0], in_=qt_d[0])
            vt_tiles = {}

            def dma_vt(kb):
                t = vtpool.tile([128, NC_CHUNKS, 128], f16, name="vt_t",
                                tag="vtt")
                nc.sync.dma_start(out=t[:], in_=vt_d[kb])
                vt_tiles[kb] = t

            nc.sync.dma_start(out=wv_sb[:], in_=wv_d[:])
            dma_vt(0)
            dma_vt(1)

            dma_done = {"kt": 1, "qt": 1}

            def dma_x(which, n):
                sb, dr = (kt_sb, kt_d) if which == "kt" else (qt_sb, qt_d)
                nc.sync.dma_start(out=sb[:, n], in_=dr[n])
                dma_done[which] = n + 1

            # ---- projection building blocks ----
            def vproj_unit(kb):
                if kb >= 2:
                    dma_vt(kb)          # prefetch handled by pool bufs=4
                vt_t = vt_tiles[kb]
                nc.vector.memset(v_sb[:, kb, :, :], 1.0)
                ps = psp.tile([128, GCOLS], f32, name="vproj_ps", tag="pps",
                              bufs=2)
                for c in range(NC_CHUNKS):
                    nc.tensor.matmul(ps[:], vt_t[:, c, :], wv_sb[:, c, :],
                                     start=(c == 0), stop=(c == NC_CHUNKS - 1))
                nc.vector.tensor_copy(
                    v_sb[:, kb, :, 0:64],
                    ps[:].rearrange("p (h d) -> p h d", h=HPC))
                vt_tiles[kb] = None     # allow pool slot reuse

            def proj_half(w_sb, xt_sb, dst, m, n, half, holder):
                if half == 0:
                    holder[0] = psp.tile([128, QB], f32, name="proj_ps",
                                         tag="pps", bufs=2)
                ps = holder[0]
                for c in range(4 * half, 4 * half + 4):
                    nc.tensor.matmul(
                        ps[:], w_sb[:, c, m * 128:(m + 1) * 128],
                        xt_sb[:, n, c, :],
                        start=(c == 0), stop=(c == NC_CHUNKS - 1))
                if half == 1:
                    nc.vector.tensor_copy(dst[:, m, n * QB:(n + 1) * QB],
                                          ps[:])

            # ---- feed list: (deadline_iter, emit_fn) ----
            # iteration index = ((j*NQB)+qb)*NKB + kb over the attention loop
            def it_idx(j, qb, kb):
                return (j * NQB + qb) * NKB + kb

            feed = []

            def add_feed(deadline, fn):
                feed.append([deadline, fn])

            # v-projections: v[kb] needed at iter (0,0,kb); emitted from the
            # feed (after the first scores) so the exp stream starts ASAP
            for kb in range(NKB):
                add_feed(max(0, it_idx(0, 0, kb) - 2),
                         (lambda kb=kb: vproj_unit(kb)))
            # kT(m, n): needed by scores(j=m, qb=0, kb=4n); qT(m, qb) at
            # (m, qb, 0). Emitted as two half-units each, with targets
            # SPREAD across earlier iterations so the PE queue never gets a
            # burst of projection work in front of the score matmuls.
            units = []
            for m in range(4):
                for n in range(NQB):
                    if not (m == 0 and n == 0):
                        if m == 0:
                            tgt = max(1, 4 * (n - 1))
                        else:
                            # spread pair-m kT units over pair m-1 qb 1..2
                            tgt = it_idx(m - 1, 1, 0) + 6 * n
                        units.append(("kt", kT_sb, wk_sb, m, n,
                                      tgt, it_idx(m, 0, 4 * n) - 3))
                    if not (m == 0 and n == 0):
                        if n == 0:
                            tgt = it_idx(m - 1, 2, 8) + 4
                        else:
                            tgt = it_idx(m, n - 1, 6)
                        units.append(("qt", qT_sb, wq_sb, m, n,
                                      tgt, it_idx(m, n, 0) - 3))
            for which, dst, w_sb, m, n, tgt, dl in units:
                holder = [None]
                xt_sb = kt_sb if which == "kt" else qt_sb

                def mk(half, which=which, dst=dst, w_sb=w_sb, m=m, n=n,
                       xt_sb=xt_sb, holder=holder):
                    def fn():
                        if dma_done[which] <= n:
                            for nn in range(dma_done[which], n + 1):
                                dma_x(which, nn)
                        proj_half(w_sb, xt_sb, dst, m, n, half, holder)
                    return fn
                add_feed(min(tgt, dl - 1), mk(0))
                add_feed(min(tgt + 1, dl), mk(1))
            feed.sort(key=lambda e: e[0])
            # remaining input DMAs are pulled in by deadline; wo early on
            wo_loaded = [False]

            def load_wo():
                if not wo_loaded[0]:
                    nc.sync.dma_start(out=wo_sb[:], in_=wo_d[:])
                    wo_loaded[0] = True

            def pump(cur_iter, budget=1, horizon=16):
                # emit overdue units, plus up to `budget` units that come
                # due within `horizon` iterations (keeps filler spread out)
                while feed and feed[0][0] <= cur_iter:
                    feed.pop(0)[1]()
                while budget > 0 and feed and feed[0][0] <= cur_iter + horizon:
                    feed.pop(0)[1]()
                    budget -= 1
                if cur_iter >= it_idx(1, 2, 0):
                    load_wo()

            # ---- prefix projections ----
            h0 = [None]
            proj_half(wk_sb, kt_sb, kT_sb, 0, 0, 0, h0)
            proj_half(wk_sb, kt_sb, kT_sb, 0, 0, 1, h0)
            h1 = [None]
            proj_half(wq_sb, qt_sb, qT_sb, 0, 0, 0, h1)
            proj_half(wq_sb, qt_sb, qT_sb, 0, 0, 1, h1)

            # ---- output projection (per 128-token tile) ----
            def y_unit(t):
                # emitted inside DVE-exp iterations: the Act engine is
                # exp-idle there, so these copies don't delay the exp chain
                for n2 in range(2):
                    y_sb = workpool.tile([128, QB], f16, name="y_sb", tag="y",
                                         bufs=2)
                    ps = psp.tile([128, QB], f32, name="y_ps", tag="pps",
                                  bufs=2)
                    for c2 in range(4):
                        nc.tensor.matmul(
                            ps[:], oT_sb[:, c2, t * 128:(t + 1) * 128],
                            wo_sb[:, c2, n2 * QB:(n2 + 1) * QB],
                            start=(c2 == 0), stop=(c2 == 3))
                    nc.scalar.copy(y_sb[:], ps[:])
                    nc.sync.dma_start(
                        out=yp_d[t * 128:(t + 1) * 128,
                                 n2 * QB:(n2 + 1) * QB],
                        in_=y_sb[:])

            # ---- attention (software-pipelined: scores/exp run one
            #      iteration ahead of attn@V so the PE never waits on exp) --
            y_pending = []
            for j in range(4):
                h0i, h1i = 2 * j, 2 * j + 1
                unnorm0 = workpool.tile([64, NQB, QB], f16, name="unnorm0",
                                        tag="unnorm0", bufs=1)
                unnorm1 = workpool.tile([64, NQB, QB], f16, name="unnorm1",
                                        tag="unnorm1", bufs=1)
                deferred = [None]
                outs = {}
                prev = [None]

                def scores_emit(qb, kb, j=j):
                    sb2 = psp.tile([128, 2, QB], f32, name="sb2",
                                   tag="sbig", bufs=2)
                    nc.tensor.matmul(
                        sb2[:, 0, :],
                        kT_sb[0:64, j, kb * 128:(kb + 1) * 128],
                        qT_sb[0:64, j, qb * QB:(qb + 1) * QB],
                        start=True, stop=True)
                    nc.tensor.matmul(
                        sb2[:, 1, :],
                        kT_sb[64:128, j, kb * 128:(kb + 1) * 128],
                        qT_sb[64:128, j, qb * QB:(qb + 1) * QB],
                        start=True, stop=True)
                    return sb2

                def exp_emit(kb, sb2):
                    pT = workpool.tile([128, 2, QB], f16, name="pT",
                                       tag="pT", bufs=4)
                    if kb in DVE_KB:
                        # Schraudolph 2^u: int16 bits of the fp16 result
                        nc.vector.tensor_scalar(
                            out=pT[:].rearrange("p a b -> p (a b)")
                                     .bitcast(i16),
                            in0=sb2[:].rearrange("p a b -> p (a b)"),
                            scalar1=1024.0, scalar2=SCHRAU_BIAS,
                            op0=MULT, op1=ADD)
                    else:
                        nc.scalar.activation(
                            pT[:].rearrange("p a b -> p (a b)"),
                            sb2[:].rearrange("p a b -> p (a b)"),
                            EXP, scale=LN2)
                    return pT

                def stage1(qb, kb, j=j):
                    return exp_emit(kb, scores_emit(qb, kb))

                def stage2(qb, kb, pT, j=j, h0i=h0i, h1i=h1i):
                    if kb == 0:
                        outs[qb] = (
                            psp.tile([128, QB], f32, name="out0", tag="out0",
                                     bufs=1),
                            psp.tile([128, QB], f32, name="out1", tag="out1",
                                     bufs=1))
                    out0, out1 = outs[qb]
                    nc.tensor.matmul(
                        out0[0:65, :], v_sb[:, kb, h0i, 0:65], pT[:, 0, :],
                        start=(kb == 0), stop=(kb == NKB - 1))
                    nc.tensor.matmul(
                        out1[0:65, :], v_sb[:, kb, h1i, 0:65], pT[:, 1, :],
                        start=(kb == 0), stop=(kb == NKB - 1))

                def qb_epilogue(qb, j=j):
                    # stage to SBUF fast (frees the PSUM accumulators);
                    # denominator rows go via the Act engine (it has slack),
                    # normalize is deferred one qb so copies never stall
                    out0, out1 = outs.pop(qb)
                    db = workpool.tile([1, 2, QB], f32, name="db", tag="db",
                                       bufs=2)
                    nc.vector.tensor_copy(db[:, 0, :], out0[64:65, :])
                    nc.vector.tensor_copy(db[:, 1, :], out1[64:65, :])
                    nc.vector.tensor_copy(unnorm0[:, qb, :], out0[0:64, :])
                    nc.vector.tensor_copy(unnorm1[:, qb, :], out1[0:64, :])

                    def _normalize(qb=qb, db=db, j=j):
                        rcp = workpool.tile([1, 2, QB], f32, name="rcp",
                                            tag="rcp", bufs=1)
                        if FAST_RECIP:
                            nc.vector.reciprocal_approx_fast(out=rcp[:],
                                                             in_=db[:])
                        else:
                            nc.vector.reciprocal(rcp[:], db[:])
                        rcph = workpool.tile([1, 2, QB], f16, name="rcph",
                                             tag="rcph", bufs=2)
                        nc.vector.tensor_copy(rcph[:], rcp[:])
                        rbc = workpool.tile([64, 2, QB], f16, name="rbc",
                                            tag="rbc", bufs=1)
                        nc.gpsimd.partition_broadcast(rbc[:, 0, :],
                                                      rcph[0:1, 0, :])
                        nc.gpsimd.partition_broadcast(rbc[:, 1, :],
                                                      rcph[0:1, 1, :])
                        nc.vector.tensor_mul(
                            oT_sb[0:64, j, qb * QB:(qb + 1) * QB],
                            unnorm0[0:64, qb, :], rbc[:, 0, :])
                        nc.vector.tensor_mul(
                            oT_sb[64:128, j, qb * QB:(qb + 1) * QB],
                            unnorm1[0:64, qb, :], rbc[:, 1, :])
                        if j == 3:
                            y_pending.extend(range(4 * qb, 4 * qb + 4))

                    if deferred[0] is not None:
                        deferred[0]()
                    deferred[0] = _normalize

                # kb iterations run in batches of 2 (fewer PE tile-mode
                # switches; the DVE-exp of kb2+1 overlaps the Act exp of kb2)
                def flush_prev():
                    for pqb, pkb, ppT in prev[0]:
                        stage2(pqb, pkb, ppT)
                        if pkb == NKB - 1:
                            qb_epilogue(pqb)
                    prev[0] = []

                prev[0] = []
                for qb in range(NQB):
                    for kb2 in range(0, NKB, 2):
                        sb2a = scores_emit(qb, kb2)
                        sb2b = scores_emit(qb, kb2 + 1)
                        pTa = exp_emit(kb2, sb2a)
                        pTb = exp_emit(kb2 + 1, sb2b)
                        if kb2 + 1 in DVE_KB and y_pending:
                            # Act engine is exp-idle this half-batch
                            y_unit(y_pending.pop(0))
                        cur = [(qb, kb2, pTa), (qb, kb2 + 1, pTb)]
                        flush_prev()
                        prev[0] = cur
                        pump(it_idx(j, qb, kb2 + 1))
                flush_prev()
                deferred[0]()
                while j == 3 and feed:
                    feed.pop(0)[1]()
            while y_pending:
                y_unit(y_pending.pop(0))

    nc.compile()
    _prog_cache[key] = nc
    return nc


def _chunk_pT_nblk(x):
    """[S, D] -> [4, 128, 8, 512] fp16: out[n, p, c, t] = x[512n+t, 128c+p]."""
    return np.ascontiguousarray(
        x.reshape(NQB, QB, NC_CHUNKS, 128).transpose(0, 3, 2, 1))


def _chunk_pT_kb(x):
    """[S, D] -> [16, 128, 8, 128]: out[k, p, c, t] = x[128k+t, 128c+p]."""
    return np.ascontiguousarray(
        x.reshape(NKB, 128, NC_CHUNKS, 128).transpose(0, 3, 2, 1))


def _chunk_w(w):
    """[D, GCOLS] -> [128, 8, GCOLS]: out[p, c, m] = w[128c+p, m]."""
    return np.ascontiguousarray(
        w.reshape(NC_CHUNKS, 128, w.shape[1]).transpose(1, 0, 2))


def prepare_in_maps(Q, K, V, W_Q, W_K, W_V, W_O):
    f16 = np.float16
    wq_scaled = (W_Q.astype(np.float32) * np.float32(LOG2E / 8.0))
    qt = [_chunk_pT_nblk(Q[b].astype(f16)) for b in range(B)]
    kt = [_chunk_pT_nblk(K[b].astype(f16)) for b in range(B)]
    vt = [_chunk_pT_kb(V[b].astype(f16)) for b in range(B)]
    wq = [_chunk_w(wq_scaled[:, g * GCOLS:(g + 1) * GCOLS].astype(f16))
          for g in range(2)]
    wk = [_chunk_w(W_K[:, g * GCOLS:(g + 1) * GCOLS].astype(f16))
          for g in range(2)]
    wv = [_chunk_w(W_V[:, g * GCOLS:(g + 1) * GCOLS].astype(f16))
          for g in range(2)]
    wo = [np.ascontiguousarray(
        W_O[g * GCOLS:(g + 1) * GCOLS, :].astype(f16)
        .reshape(4, 128, D).transpose(1, 0, 2)) for g in range(2)]
    in_maps = []
    for c in range(NCORES):
        b, g = c // 2, c % 2
        in_maps.append({
            "qt": qt[b], "kt": kt[b], "vt": vt[b],
            "wq": wq[g], "wk": wk[g], "wv": wv[g], "wo": wo[g],
        })
    return in_maps


def execute(nc, in_maps):
    from concourse.bass_utils import run_bass_kernel_spmd
    res = run_bass_kernel_spmd(nc, in_maps, list(range(NCORES)))
    return res


def _numpy_fallback(Q, K, V, mask, W_Q, W_K, W_V, W_O):
    B_, S1, _ = Q.shape
    q = (Q.reshape(-1, D) @ W_Q).reshape(B_, S1, H, DK).transpose(0, 2, 1, 3)
    k = (K.reshape(-1, D) @ W_K).reshape(B_, S1, H, DK).transpose(0, 2, 1, 3)
    v = (V.reshape(-1, D) @ W_V).reshape(B_, S1, H, DK).transpose(0, 2, 1, 3)
    out = np.empty((B_, H, S1, DK), np.float32)
    for b in range(B_):
        for h in range(H):
            s = (q[b, h] @ k[b, h].T) / math.sqrt(DK)
            s = np.where(mask[b] == 0, np.float32(-1e9), s)
            s = s - s.max(axis=-1, keepdims=True)
            e = np.exp(s)
            p = e / e.sum(axis=-1, keepdims=True)
            out[b, h] = p @ v[b, h]
    o = out.transpose(0, 2, 1, 3).reshape(B_, S1, D)
    return (o.reshape(-1, D) @ W_O).reshape(B_, S1, D).astype(np.float32)


def kernel(Q, K, V, mask, W_Q, W_K, W_V, W_O):
    Q = np.asarray(Q); K = np.asarray(K); V = np.asarray(V)
    mask = np.asarray(mask)
    W_Q = np.asarray(W_Q); W_K = np.asarray(W_K)
    W_V = np.asarray(W_V); W_O = np.asarray(W_O)
    if (mask == 0).any():
        # spec guarantees an all-ones mask; this path is correctness insurance
        return _numpy_fallback(Q, K, V, mask, W_Q, W_K, W_V, W_O)
    nc = build_program()
    in_maps = prepare_in_maps(Q, K, V, W_Q, W_K, W_V, W_O)
    res = execute(nc, in_maps)
    out = np.empty((B, S, D), np.float32)
    for b in range(B):
        out[b] = (res.results[2 * b]["yp"].astype(np.float32)
                  + res.results[2 * b + 1]["yp"].astype(np.float32))
    return out
